# revision 11
# baseline (speedup 1.0000x reference)
"""Trainium2 Bass kernel for the vq_codebook problem.

Computes, per batch b (B=32, d=512, n=4096, r=64, T=10, 3 steps):
    D = normalize(D_init, dim=d)
    repeat 3x: Dn = normalize(D); cos = Dn^T @ normalize(X, dim=d);
               C = softmax(cos / T, over r); D = X @ C^T   (normalize-invariant
               scale factors like the per-codeword count division cancel)
    Xbar = normalize(D) @ C of the last step.

Sharding: pure batch parallelism, 4 batches per NeuronCore across 8 cores.

Layout strategy (per core):
  - Host uploads X twice in the layouts the PE wants: XT = X^T in bf16
    (contraction over n for X@C^T, also the 1/||x_n|| source) and X8 = X
    in fp8-e4m3 packed in d-tile pairs (stationary operand of the cos
    matmul, DoubleRow mode).  No on-device transposes or casts of X.
  - n is chunked p-major (n = p*32 + c): chunk c covers n = {p*32+c},
    so every per-n quantity (logit scale, softmax denominator) is a
    per-partition vector for that chunk, and the softmax exp can run
    straight off the cos PSUM as one fused ACT op per chunk with a
    per-partition scale and a free running-sum accumulator.
  - cos^T lands in [n-partitions, r-free] via fp8 DoubleRow matmuls with
    full 128-partition outputs; X@C^T runs in the natural [d-partitions,
    r] layout, then a cheap bf16 transpose gives D^T for the
    column-normalize (a free-dim reduction there).
  - Two batches are software-pipelined step-by-step so every engine
    queue interleaves two independent dependency chains.
  - Y is produced in bf16 (copies on the otherwise-idle GPSIMD engine)
    and upcast to f32 on the host after the gather.
"""

import contextlib
import math

import numpy as np

import concourse.bacc as bacc
import concourse.bass as bass
import concourse.mybir as mybir
import concourse.tile as tile
from concourse.bass_utils import run_bass_kernel_spmd

F32 = mybir.dt.float32
BF16 = mybir.dt.bfloat16
F8 = mybir.dt.float8e4
AF = mybir.ActivationFunctionType
OP = mybir.AluOpType
DR = mybir.MatmulPerfMode.DoubleRow

N_CORES = 8
B_FULL, D, N, R = 32, 512, 4096, 64
B_LOC = B_FULL // N_CORES          # 4 batches per core
KT = D // 128                      # 4 d-tiles
NC = N // 128                      # 32 n-chunks of 128 (p-major: n=p*32+c)
T_INV = 0.1                        # 1 / temperature
LN_TINV = math.log(T_INV)
STEPS = 3
EPS2 = 1e-12                       # eps^2 for the norm clamp


def _rsqrt_clamped(nc, pool, src_ap, p, name, eps_t, bias2=None):
    """exp(-0.5 * ln(src + EPS2) [+ bias2]) as a [p, 1] tile."""
    m = src_ap.shape[1]
    ln = pool.tile([p, m], F32, tag=f"{name}_ln")
    nc.scalar.activation(out=ln, in_=src_ap, func=AF.Ln, scale=1.0,
                         bias=eps_t[:p, 0:1])
    rs = pool.tile([p, m], F32, tag=f"{name}_rs")
    nc.scalar.activation(out=rs, in_=ln, func=AF.Exp, scale=-0.5,
                         bias=0.0 if bias2 is None else bias2[:p, 0:1])
    return rs


def _force_single_act_set():
    """All ACT functions we use (Exp, Ln, Square, Copy) live in the
    natural_log_exp_and_others set.  Empty out every other set so a single
    table load suffices."""
    import concourse.hw_specs as hw_specs

    orig = hw_specs.get_activation_tables
    target = "natural_log_exp_and_others"

    def patched(arch):
        t = dict(orig(arch))
        need = {AF.Exp, AF.Ln, AF.Square, AF.Copy}
        if target in t and need <= set(t[target]):
            t = {k: (v if k == target else set()) for k, v in t.items()}
        return t

    bacc.get_activation_tables = patched


def build_program():
    _force_single_act_set()
    nc = bacc.Bacc()
    # X^T, p-major n rows: XT[b, p, c, d] = X[b, d, p*32+c], bf16
    xt_ext = nc.declare_dram_parameter("XT", [B_LOC, 128, NC, D], BF16,
                                       isOutput=False)
    # X natural fp8, d-tiles packed in pairs: [kp, p, t, n], d=kp*256+t*128+p
    x8_ext = nc.declare_dram_parameter("X8", [B_LOC, 2, 128, 2, N], F8,
                                       isOutput=False)
    # D_init^T: [r, d] bf16 (host pre-transposed)
    dt_ext = nc.declare_dram_parameter("DT", [B_LOC, R, D], BF16,
                                       isOutput=False)
    id_ext = nc.declare_dram_parameter("ident", [128, 128], BF16,
                                       isOutput=False)
    y_ext = nc.declare_dram_parameter("Y", [B_LOC, D, N], BF16, isOutput=True)

    with tile.TileContext(nc) as tc:
        with contextlib.ExitStack() as ctx:
            singles = ctx.enter_context(tc.tile_pool(name="singles", bufs=1))
            xpool = ctx.enter_context(tc.tile_pool(name="xpool", bufs=2))
            work = ctx.enter_context(tc.tile_pool(name="work", bufs=2))
            ypool = ctx.enter_context(tc.tile_pool(name="ypool", bufs=4))
            ps_cos = ctx.enter_context(
                tc.tile_pool(name="ps_cos", bufs=2, space="PSUM"))
            ps_m = ctx.enter_context(
                tc.tile_pool(name="ps_m", bufs=2, space="PSUM"))
            ps_out = ctx.enter_context(
                tc.tile_pool(name="ps_out", bufs=2, space="PSUM"))

            id_b = singles.tile([128, 128], BF16)
            nc.sync.dma_start(out=id_b, in_=id_ext[:])
            eps_t = singles.tile([128, 1], F32)
            nc.vector.memset(eps_t, EPS2)
            lnt_t = singles.tile([128, 1], F32)
            nc.vector.memset(lnt_t, LN_TINV)

            def emit_loads(b):
                """Input DMAs for batch b; returns a state dict."""
                xt = xpool.tile([128, NC, D], BF16, tag="xt", name=f"xt{b}",
                                bufs=3)
                nc.sync.dma_start(out=xt, in_=xt_ext[b, :, :, :])
                x8 = []
                for kp in range(2):
                    t = xpool.tile([128, 2, N], F8, tag=f"x8_{kp}",
                                   name=f"x8_{b}_{kp}")
                    nc.sync.dma_start(out=t, in_=x8_ext[b, kp])
                    x8.append(t)
                dt0 = xpool.tile([64, D], BF16, tag="dt0", name=f"dt0_{b}")
                nc.sync.dma_start(out=dt0, in_=dt_ext[b])
                return {"xt": xt, "x8": x8, "dt": dt0}

            def emit_prep(b, st):
                """ssq -> scl2 (logit scales) for batch b."""
                xt = st["xt"]
                ssq = work.tile([128, NC], F32, tag="ssq")
                sq_scr = work.tile([128, 8, D], BF16, tag="sqscr", bufs=1)
                for c8 in range(4):
                    nc.vector.tensor_tensor(
                        out=sq_scr, in0=xt[:, 8 * c8:8 * (c8 + 1), :],
                        in1=xt[:, 8 * c8:8 * (c8 + 1), :], op=OP.mult,
                    )
                    for j in range(8):
                        c = 8 * c8 + j
                        nc.vector.tensor_scalar(
                            out=sq_scr[:, j, :], in0=sq_scr[:, j, :],
                            scalar1=1.0, scalar2=None, op0=OP.mult,
                            accum_out=ssq[:, c:c + 1],
                        )
                ln_x = work.tile([128, NC], F32, tag="lnx")
                nc.scalar.activation(out=ln_x, in_=ssq, func=AF.Ln,
                                     scale=1.0, bias=eps_t[:, 0:1])
                scl2 = work.tile([128, NC], F32, tag="scl2")
                nc.scalar.activation(out=scl2, in_=ln_x, func=AF.Exp,
                                     scale=-0.5, bias=lnt_t[:, 0:1])
                st["scl2"] = scl2

            def emit_step(b, s, st):
                """One VQ refinement step for batch b."""
                xt, x8, scl2 = st["xt"], st["x8"], st["scl2"]
                dt_cur = st["dt"]
                last = s == STEPS - 1

                # normalize D columns (rows of D^T) -> dn8 (fp8, natural)
                ssqd = work.tile([64, 1], F32, tag="ssqd")
                dsq_scr = work.tile([64, D], BF16, tag="dsqscr", bufs=1)
                nc.scalar.activation(
                    out=dsq_scr, in_=dt_cur, func=AF.Square, scale=1.0,
                    bias=0.0, accum_out=ssqd,
                )
                rnd = _rsqrt_clamped(nc, work, ssqd[:, :], 64, "rnd", eps_t)
                dnt = work.tile([64, D], BF16, tag="dnt")
                nc.scalar.activation(out=dnt, in_=dt_cur, func=AF.Copy,
                                     scale=rnd[:, 0:1])
                pdn = ps_m.tile([128, KT, R], BF16, tag="psmall")
                for k in range(KT):
                    nc.tensor.transpose(
                        pdn[:, k, :], dnt[:, k * 128:(k + 1) * 128],
                        id_b[0:64, 0:64])
                dn8 = work.tile([128, KT, R], F8, tag="dn8")
                nc.vector.tensor_copy(out=dn8, in_=pdn)

                # cos^T chunks + fused softmax pass 1 (exp with per-partition
                # scale, running-sum accumulator), then ct = et * (1/s)
                s_sum = work.tile([128, NC], F32, tag="ssum")
                et = work.tile([128, NC, R], BF16, tag="et")
                ct = work.tile([128, NC, R], BF16, tag="ct")
                rs = work.tile([128, NC], F32, tag="rs")
                pcs = []
                for g in range(4):
                    pc = ps_cos.tile([128, 8, R], F32, tag="pcos")
                    pcs.append(pc)
                    for j in range(8):
                        c = g * 8 + j
                        x8col = [
                            bass.AP(
                                tensor=t.tensor, offset=t.offset + c,
                                ap=[list(t.ap[0]), [N, 2], [NC, 128]],
                            ) for t in x8
                        ]
                        for kp in range(2):
                            nc.tensor.matmul(
                                pc[:, j, :], x8col[kp],
                                dn8[:, 2 * kp:2 * kp + 2, :],
                                start=(kp == 0), stop=(kp == 1),
                                perf_mode=DR,
                            )
                        nc.scalar.activation(
                            out=et[:, c, :], in_=pc[:, j, :], func=AF.Exp,
                            scale=scl2[:, c:c + 1], bias=0.0,
                            accum_out=s_sum[:, c:c + 1],
                        )
                nc.vector.reciprocal(out=rs, in_=s_sum)
                for c in range(NC):
                    nc.vector.tensor_scalar(
                        out=ct[:, c, :], in0=et[:, c, :],
                        scalar1=rs[:, c:c + 1], scalar2=None, op0=OP.mult,
                    )

                if s == 0:
                    st["post_s0"]()

                # XCt = X @ C^T in natural [d128, r] layout
                pxct = ps_m.tile([128, KT, R], F32, tag="psmall")
                for dd in range(KT):
                    for c in range(NC):
                        nc.tensor.matmul(
                            pxct[:, dd, :],
                            xt[:, c, dd * 128:(dd + 1) * 128],
                            ct[:, c, :],
                            start=(c == 0), stop=(c == NC - 1),
                        )
                xct_n = work.tile([128, KT, R], BF16, tag="xctn")
                nc.vector.tensor_copy(out=xct_n, in_=pxct)

                # D^T of the new codebook
                pdt = ps_m.tile([64, D], BF16, tag="pdt")
                for dd in range(KT):
                    nc.tensor.transpose(
                        pdt[:, dd * 128:(dd + 1) * 128], xct_n[:, dd, :],
                        id_b)
                if not last:
                    st["dt"] = pdt
                else:
                    st["ct"] = ct
                    # Dnew = normalize(XCt), bf16, for the Xbar matmul
                    ssqf = work.tile([64, 1], F32, tag="ssqf")
                    fsq_scr = work.tile([64, D], BF16, tag="fsqscr", bufs=1)
                    nc.scalar.activation(
                        out=fsq_scr, in_=pdt, func=AF.Square, scale=1.0,
                        bias=0.0, accum_out=ssqf,
                    )
                    rnf = _rsqrt_clamped(nc, work, ssqf[:, :], 64, "rnf",
                                         eps_t)
                    dnew_b = work.tile([64, D], BF16, tag="dnewb")
                    nc.scalar.activation(out=dnew_b, in_=pdt, func=AF.Copy,
                                         scale=rnf[:, 0:1])
                    st["dnew"] = dnew_b

            def emit_output(b, st):
                """C natural, Xbar = Dnew @ C, Y copies + DMAs."""
                ct, dnew_b = st["ct"], st["dnew"]
                # C natural [r, n]: transpose ct chunks; chunk c scatters to
                # columns n = p*32 + c
                c_r = work.tile([64, N], BF16, tag="c_r", bufs=1)
                for q in range(4):
                    pcr = ps_cos.tile([64, 8, 128], BF16, tag="pcos")
                    for j in range(8):
                        nc.tensor.transpose(
                            pcr[:, j, :], ct[:, q * 8 + j, :], id_b)
                    dst = bass.AP(
                        tensor=c_r.tensor, offset=c_r.offset + 8 * q,
                        ap=[list(c_r.ap[0]), [1, 8], [NC, 128]],
                    )
                    if q % 2 == 0:
                        nc.scalar.copy(out=dst, in_=pcr)
                    else:
                        nc.vector.tensor_copy(out=dst, in_=pcr)
                # Xbar = Dnew @ C -> bf16 staging -> DRAM (4 tiles per DMA)
                for k in range(KT):
                    for half in range(2):
                        ot = ypool.tile([128, 4, 512], BF16, tag="osb")
                        for q in range(4):
                            j = half * 4 + q
                            pxb = ps_out.tile([128, 512], F32, tag="pxb")
                            nc.tensor.matmul(
                                pxb, dnew_b[:, k * 128:(k + 1) * 128],
                                c_r[:, j * 512:(j + 1) * 512],
                                start=True, stop=True,
                            )
                            nc.gpsimd.tensor_copy(out=ot[:, q, :], in_=pxb)
                        nc.sync.dma_start(
                            out=y_ext[b, k * 128:(k + 1) * 128,
                                      half * 2048:(half + 1) * 2048],
                            in_=ot,
                        )

            # ---- two-batch software pipeline --------------------------------
            states = {}
            for pair0 in range(0, B_LOC, 2):
                pair = [pair0, pair0 + 1]
                for b in pair:
                    if b not in states:
                        states[b] = emit_loads(b)

                def make_post_s0(pair0=pair0):
                    done = {"v": False}

                    def post_s0():
                        # prefetch the next pair's inputs after the first
                        # steps, ahead of this pair's output DMAs
                        if not done["v"] and pair0 + 2 < B_LOC:
                            done["v"] = True
                            states[pair0 + 2] = emit_loads(pair0 + 2)
                            states[pair0 + 3] = emit_loads(pair0 + 3)

                    return post_s0

                post = make_post_s0()
                for b in pair:
                    states[b]["post_s0"] = post
                for b in pair:
                    emit_prep(b, states[b])
                for s in range(STEPS):
                    for b in pair:
                        emit_step(b, s, states[b])
                for b in pair:
                    emit_output(b, states[b])
                    del states[b]
    nc.finalize()
    return nc


_NC_CACHE = None
_last_in_maps = None


def kernel(X: np.ndarray, D_init: np.ndarray) -> np.ndarray:
    global _NC_CACHE, _last_in_maps
    import ml_dtypes

    X = np.asarray(X, dtype=np.float32)
    D_init = np.asarray(D_init, dtype=np.float32)
    if _NC_CACHE is None:
        _NC_CACHE = build_program()
    nc = _NC_CACHE
    ident = np.eye(128, dtype=ml_dtypes.bfloat16)
    # XT[b, p, c, d] = X[b, d, p*32+c]
    xt_h = np.ascontiguousarray(
        X.transpose(0, 2, 1).reshape(B_FULL, 128, NC, D)
    ).astype(ml_dtypes.bfloat16)
    # X8[b, kp, p, t, n] = X[b, kp*256+t*128+p, n]
    x8_h = np.ascontiguousarray(
        X.reshape(B_FULL, 2, 2, 128, N).transpose(0, 1, 3, 2, 4)
    ).astype(ml_dtypes.float8_e4m3)
    dt_h = np.ascontiguousarray(
        D_init.transpose(0, 2, 1)).astype(ml_dtypes.bfloat16)
    in_maps = [
        {
            "XT": np.ascontiguousarray(xt_h[i * B_LOC:(i + 1) * B_LOC]),
            "X8": np.ascontiguousarray(x8_h[i * B_LOC:(i + 1) * B_LOC]),
            "DT": np.ascontiguousarray(dt_h[i * B_LOC:(i + 1) * B_LOC]),
            "ident": ident,
        }
        for i in range(N_CORES)
    ]
    _last_in_maps = in_maps
    res = run_bass_kernel_spmd(nc, in_maps, list(range(N_CORES)))
    out = np.concatenate(
        [np.asarray(res.results[i]["Y"]) for i in range(N_CORES)], axis=0)
    return out.astype(np.float32)


# revision 15
# speedup vs baseline: 1.1872x; 1.1872x over previous
"""Trainium2 Bass kernel for the vq_codebook problem.

Computes, per batch b (B=32, d=512, n=4096, r=64, T=10, 3 steps):
    D = normalize(D_init, dim=d)
    repeat 3x: Dn = normalize(D); cos = Dn^T @ normalize(X, dim=d);
               C = softmax(cos / T, over r); D = X @ C^T   (normalize-invariant
               scale factors like the per-codeword count division cancel)
    Xbar = normalize(D) @ C of the last step.

Sharding: pure batch parallelism, 4 batches per NeuronCore across 8 cores.

Layout strategy (per core):
  - Host uploads X twice in the layouts the PE wants: XT = X^T in bf16
    (contraction over n for X@C^T, also the 1/||x_n|| source) and X8 = X
    in fp8-e4m3 packed in d-tile pairs (stationary operand of the cos
    matmul, DoubleRow mode).  No on-device transposes or casts of X.
  - n is chunked p-major (n = p*32 + c): chunk c covers n = {p*32+c},
    so every per-n quantity (logit scale, softmax denominator) is a
    per-partition vector for that chunk, and the softmax exp can run
    straight off the cos PSUM as one fused ACT op per chunk with a
    per-partition scale and a free running-sum accumulator.
  - cos^T lands in [n-partitions, r-free] via fp8 DoubleRow matmuls with
    full 128-partition outputs; X@C^T runs in the natural [d-partitions,
    r] layout, then a cheap bf16 transpose gives D^T for the
    column-normalize (a free-dim reduction there).
  - Two batches are software-pipelined step-by-step so every engine
    queue interleaves two independent dependency chains.
  - Y is produced in bf16 (copies on the otherwise-idle GPSIMD engine)
    and upcast to f32 on the host after the gather.
"""

import contextlib
import math

import numpy as np

import concourse.bacc as bacc
import concourse.bass as bass
import concourse.mybir as mybir
import concourse.tile as tile
from concourse.bass_utils import run_bass_kernel_spmd

F32 = mybir.dt.float32
BF16 = mybir.dt.bfloat16
F8 = mybir.dt.float8e4
AF = mybir.ActivationFunctionType
OP = mybir.AluOpType
DR = mybir.MatmulPerfMode.DoubleRow

N_CORES = 8
B_FULL, D, N, R = 32, 512, 4096, 64
B_LOC = B_FULL // N_CORES          # 4 batches per core
KT = D // 128                      # 4 d-tiles
NC = N // 128                      # 32 n-chunks of 128 (p-major: n=p*32+c)
T_INV = 0.1                        # 1 / temperature
LN_TINV = math.log(T_INV)
STEPS = 3
EPS2 = 1e-12                       # eps^2 for the norm clamp


def _rsqrt_clamped(nc, pool, src_ap, p, name, eps_t, bias2=None):
    """exp(-0.5 * ln(src + EPS2) [+ bias2]) as a [p, 1] tile."""
    m = src_ap.shape[1]
    ln = pool.tile([p, m], F32, tag=f"{name}_ln")
    nc.scalar.activation(out=ln, in_=src_ap, func=AF.Ln, scale=1.0,
                         bias=eps_t[:p, 0:1])
    rs = pool.tile([p, m], F32, tag=f"{name}_rs")
    nc.scalar.activation(out=rs, in_=ln, func=AF.Exp, scale=-0.5,
                         bias=0.0 if bias2 is None else bias2[:p, 0:1])
    return rs


def _force_single_act_set():
    """All ACT functions we use (Exp, Ln, Square, Copy) live in the
    natural_log_exp_and_others set.  Empty out every other set so a single
    table load suffices."""
    import concourse.hw_specs as hw_specs

    orig = hw_specs.get_activation_tables
    target = "natural_log_exp_and_others"

    def patched(arch):
        t = dict(orig(arch))
        need = {AF.Exp, AF.Ln, AF.Square, AF.Copy}
        if target in t and need <= set(t[target]):
            t = {k: (v if k == target else set()) for k, v in t.items()}
        return t

    bacc.get_activation_tables = patched


def build_program():
    _force_single_act_set()
    nc = bacc.Bacc()
    # X^T, p-major n rows: XT[b, p, c, d] = X[b, d, p*32+c], bf16
    xt_ext = nc.declare_dram_parameter("XT", [B_LOC, 128, NC, D], BF16,
                                       isOutput=False)
    # X natural fp8, d-tiles packed in pairs: [kp, p, t, n], d=kp*256+t*128+p
    x8_ext = nc.declare_dram_parameter("X8", [B_LOC, 2, 128, 2, N], F8,
                                       isOutput=False)
    # D_init^T: [r, d] bf16 (host pre-transposed)
    dt_ext = nc.declare_dram_parameter("DT", [B_LOC, R, D], BF16,
                                       isOutput=False)
    id_ext = nc.declare_dram_parameter("ident", [128, 128], BF16,
                                       isOutput=False)
    y_ext = nc.declare_dram_parameter("Y", [B_LOC, D, N], BF16, isOutput=True)

    with tile.TileContext(nc) as tc:
        with contextlib.ExitStack() as ctx:
            singles = ctx.enter_context(tc.tile_pool(name="singles", bufs=1))
            xpool = ctx.enter_context(tc.tile_pool(name="xpool", bufs=2))
            work = ctx.enter_context(tc.tile_pool(name="work", bufs=2))
            ypool = ctx.enter_context(tc.tile_pool(name="ypool", bufs=4))
            ps_cos = ctx.enter_context(
                tc.tile_pool(name="ps_cos", bufs=2, space="PSUM"))
            ps_m = ctx.enter_context(
                tc.tile_pool(name="ps_m", bufs=2, space="PSUM"))
            ps_out = ctx.enter_context(
                tc.tile_pool(name="ps_out", bufs=2, space="PSUM"))

            id_b = singles.tile([128, 128], BF16)
            nc.sync.dma_start(out=id_b, in_=id_ext[:])
            eps_t = singles.tile([128, 1], F32)
            nc.vector.memset(eps_t, EPS2)
            lnt_t = singles.tile([128, 1], F32)
            nc.vector.memset(lnt_t, LN_TINV)

            def emit_loads(b):
                """Input DMAs for batch b; returns a state dict."""
                xt = xpool.tile([128, NC, D], BF16, tag="xt", name=f"xt{b}",
                                bufs=3)
                nc.sync.dma_start(out=xt, in_=xt_ext[b, :, :, :])
                x8 = []
                for kp in range(2):
                    t = xpool.tile([128, 2, N], F8, tag=f"x8_{kp}",
                                   name=f"x8_{b}_{kp}")
                    nc.sync.dma_start(out=t, in_=x8_ext[b, kp])
                    x8.append(t)
                dt0 = xpool.tile([64, D], BF16, tag="dt0", name=f"dt0_{b}")
                nc.sync.dma_start(out=dt0, in_=dt_ext[b])
                return {"xt": xt, "x8": x8, "dt": dt0}

            def emit_prep(b, st):
                """ssq -> scl2 (logit scales) for batch b."""
                xt = st["xt"]
                ssq = work.tile([128, NC], F32, tag="ssq")
                sq_scr = work.tile([128, 8, D], BF16, tag="sqscr", bufs=1)
                for c8 in range(4):
                    nc.vector.tensor_tensor(
                        out=sq_scr, in0=xt[:, 8 * c8:8 * (c8 + 1), :],
                        in1=xt[:, 8 * c8:8 * (c8 + 1), :], op=OP.mult,
                    )
                    for j in range(8):
                        c = 8 * c8 + j
                        nc.vector.tensor_scalar(
                            out=sq_scr[:, j, :], in0=sq_scr[:, j, :],
                            scalar1=1.0, scalar2=None, op0=OP.mult,
                            accum_out=ssq[:, c:c + 1],
                        )
                ln_x = work.tile([128, NC], F32, tag="lnx")
                nc.scalar.activation(out=ln_x, in_=ssq, func=AF.Ln,
                                     scale=1.0, bias=eps_t[:, 0:1])
                scl2 = work.tile([128, NC], F32, tag="scl2")
                nc.scalar.activation(out=scl2, in_=ln_x, func=AF.Exp,
                                     scale=-0.5, bias=lnt_t[:, 0:1])
                st["scl2"] = scl2

            def emit_step(b, s, st):
                """One VQ refinement step for batch b."""
                xt, x8, scl2 = st["xt"], st["x8"], st["scl2"]
                dt_cur = st["dt"]
                last = s == STEPS - 1

                # normalize D columns (rows of D^T) -> dn8 (fp8, natural)
                ssqd = work.tile([64, 1], F32, tag="ssqd")
                dsq_scr = work.tile([64, D], BF16, tag="dsqscr", bufs=1)
                nc.scalar.activation(
                    out=dsq_scr, in_=dt_cur, func=AF.Square, scale=1.0,
                    bias=0.0, accum_out=ssqd,
                )
                rnd = _rsqrt_clamped(nc, work, ssqd[:, :], 64, "rnd", eps_t)
                dnt = work.tile([64, D], BF16, tag="dnt")
                nc.scalar.activation(out=dnt, in_=dt_cur, func=AF.Copy,
                                     scale=rnd[:, 0:1])
                pdn = ps_m.tile([128, KT, R], BF16, tag="psmall")
                for k in range(KT):
                    nc.tensor.transpose(
                        pdn[:, k, :], dnt[:, k * 128:(k + 1) * 128],
                        id_b[0:64, 0:64])
                dn8 = work.tile([128, KT, R], F8, tag="dn8")
                nc.vector.tensor_copy(out=dn8, in_=pdn)

                # cos^T chunks -> logits (Pool) -> exp (ACT, batched) ->
                # tree-sum over r (DVE) -> ct = et * (1/s) (DVE 4x)
                lg = work.tile([128, NC, R], BF16, tag="lg")
                et = work.tile([128, NC, R], BF16, tag="et")
                ct = work.tile([128, NC, R], BF16, tag="ct")
                rs = work.tile([128, NC], F32, tag="rs")
                for g in range(4):
                    pc = ps_cos.tile([128, 8, R], F32, tag="pcos")
                    for j in range(8):
                        c = g * 8 + j
                        x8col = [
                            bass.AP(
                                tensor=t.tensor, offset=t.offset + c,
                                ap=[list(t.ap[0]), [N, 2], [NC, 128]],
                            ) for t in x8
                        ]
                        for kp in range(2):
                            nc.tensor.matmul(
                                pc[:, j, :], x8col[kp],
                                dn8[:, 2 * kp:2 * kp + 2, :],
                                start=(kp == 0), stop=(kp == 1),
                                perf_mode=DR,
                            )
                    scl_b = bass.AP(
                        tensor=scl2.tensor, offset=scl2.offset + 8 * g,
                        ap=[list(scl2.ap[0]), [1, 8], [0, R]],
                    )
                    nc.gpsimd.tensor_tensor(
                        out=lg[:, 8 * g:8 * (g + 1), :], in0=pc, in1=scl_b,
                        op=OP.mult,
                    )
                nc.scalar.activation(out=et, in_=lg, func=AF.Exp,
                                     scale=1.0, bias=0.0)
                s_sum = work.tile([128, NC], F32, tag="ssum")
                nc.vector.tensor_reduce(
                    out=s_sum, in_=et, axis=mybir.AxisListType.X, op=OP.add)
                nc.vector.reciprocal(out=rs, in_=s_sum)
                for c in range(NC):
                    nc.vector.tensor_scalar(
                        out=ct[:, c, :], in0=et[:, c, :],
                        scalar1=rs[:, c:c + 1], scalar2=None, op0=OP.mult,
                    )

                if s == 0:
                    st["post_s0"]()

                # XCt = X @ C^T in natural [d128, r] layout
                pxct = ps_m.tile([128, KT, R], F32, tag="psmall")
                for dd in range(KT):
                    for c in range(NC):
                        nc.tensor.matmul(
                            pxct[:, dd, :],
                            xt[:, c, dd * 128:(dd + 1) * 128],
                            ct[:, c, :],
                            start=(c == 0), stop=(c == NC - 1),
                        )
                xct_n = work.tile([128, KT, R], BF16, tag="xctn")
                nc.vector.tensor_copy(out=xct_n, in_=pxct)

                # D^T of the new codebook
                pdt = ps_m.tile([64, D], BF16, tag="pdt")
                for dd in range(KT):
                    nc.tensor.transpose(
                        pdt[:, dd * 128:(dd + 1) * 128], xct_n[:, dd, :],
                        id_b)
                if not last:
                    st["dt"] = pdt
                else:
                    st["ct"] = ct
                    # Dnew = normalize(XCt), bf16, for the Xbar matmul
                    ssqf = work.tile([64, 1], F32, tag="ssqf")
                    fsq_scr = work.tile([64, D], BF16, tag="fsqscr", bufs=1)
                    nc.scalar.activation(
                        out=fsq_scr, in_=pdt, func=AF.Square, scale=1.0,
                        bias=0.0, accum_out=ssqf,
                    )
                    rnf = _rsqrt_clamped(nc, work, ssqf[:, :], 64, "rnf",
                                         eps_t)
                    dnew_b = work.tile([64, D], BF16, tag="dnewb")
                    nc.scalar.activation(out=dnew_b, in_=pdt, func=AF.Copy,
                                         scale=rnf[:, 0:1])
                    st["dnew"] = dnew_b

            def emit_output(b, st):
                """C natural, Xbar = Dnew @ C, Y copies + DMAs."""
                ct, dnew_b = st["ct"], st["dnew"]
                # C natural [r, n]: transpose ct chunks; chunk c scatters to
                # columns n = p*32 + c
                c_r = work.tile([64, N], BF16, tag="c_r", bufs=1)
                for q in range(4):
                    pcr = ps_cos.tile([64, 8, 128], BF16, tag="pcos")
                    for j in range(8):
                        nc.tensor.transpose(
                            pcr[:, j, :], ct[:, q * 8 + j, :], id_b)
                    dst = bass.AP(
                        tensor=c_r.tensor, offset=c_r.offset + 8 * q,
                        ap=[list(c_r.ap[0]), [1, 8], [NC, 128]],
                    )
                    nc.scalar.copy(out=dst, in_=pcr)
                # Xbar = Dnew @ C -> bf16 staging -> DRAM (4 tiles per DMA)
                for k in range(KT):
                    for half in range(2):
                        ot = ypool.tile([128, 4, 512], BF16, tag="osb")
                        for q in range(4):
                            j = half * 4 + q
                            pxb = ps_out.tile([128, 512], F32, tag="pxb")
                            nc.tensor.matmul(
                                pxb, dnew_b[:, k * 128:(k + 1) * 128],
                                c_r[:, j * 512:(j + 1) * 512],
                                start=True, stop=True,
                            )
                            if (k * 2 + half) % 8 < 5:
                                nc.scalar.copy(out=ot[:, q, :], in_=pxb)
                            else:
                                nc.gpsimd.tensor_copy(out=ot[:, q, :],
                                                      in_=pxb)
                        nc.sync.dma_start(
                            out=y_ext[b, k * 128:(k + 1) * 128,
                                      half * 2048:(half + 1) * 2048],
                            in_=ot,
                        )

            # ---- two-batch software pipeline --------------------------------
            states = {}
            for pair0 in range(0, B_LOC, 2):
                pair = [pair0, pair0 + 1]
                for b in pair:
                    if b not in states:
                        states[b] = emit_loads(b)

                def make_post_s0(pair0=pair0):
                    done = {"v": False}

                    def post_s0():
                        # prefetch the next pair's inputs after the first
                        # steps, ahead of this pair's output DMAs
                        if not done["v"] and pair0 + 2 < B_LOC:
                            done["v"] = True
                            states[pair0 + 2] = emit_loads(pair0 + 2)
                            states[pair0 + 3] = emit_loads(pair0 + 3)

                    return post_s0

                post = make_post_s0()
                for b in pair:
                    states[b]["post_s0"] = post
                for b in pair:
                    emit_prep(b, states[b])
                for s in range(STEPS):
                    for b in pair:
                        emit_step(b, s, states[b])
                for b in pair:
                    emit_output(b, states[b])
                    del states[b]
    nc.finalize()
    return nc


_NC_CACHE = None
_last_in_maps = None


def kernel(X: np.ndarray, D_init: np.ndarray) -> np.ndarray:
    global _NC_CACHE, _last_in_maps
    import ml_dtypes

    X = np.asarray(X, dtype=np.float32)
    D_init = np.asarray(D_init, dtype=np.float32)
    if _NC_CACHE is None:
        _NC_CACHE = build_program()
    nc = _NC_CACHE
    ident = np.eye(128, dtype=ml_dtypes.bfloat16)
    # XT[b, p, c, d] = X[b, d, p*32+c]
    xt_h = np.ascontiguousarray(
        X.transpose(0, 2, 1).reshape(B_FULL, 128, NC, D)
    ).astype(ml_dtypes.bfloat16)
    # X8[b, kp, p, t, n] = X[b, kp*256+t*128+p, n]
    x8_h = np.ascontiguousarray(
        X.reshape(B_FULL, 2, 2, 128, N).transpose(0, 1, 3, 2, 4)
    ).astype(ml_dtypes.float8_e4m3)
    dt_h = np.ascontiguousarray(
        D_init.transpose(0, 2, 1)).astype(ml_dtypes.bfloat16)
    in_maps = [
        {
            "XT": np.ascontiguousarray(xt_h[i * B_LOC:(i + 1) * B_LOC]),
            "X8": np.ascontiguousarray(x8_h[i * B_LOC:(i + 1) * B_LOC]),
            "DT": np.ascontiguousarray(dt_h[i * B_LOC:(i + 1) * B_LOC]),
            "ident": ident,
        }
        for i in range(N_CORES)
    ]
    _last_in_maps = in_maps
    res = run_bass_kernel_spmd(nc, in_maps, list(range(N_CORES)))
    out = np.concatenate(
        [np.asarray(res.results[i]["Y"]) for i in range(N_CORES)], axis=0)
    return out.astype(np.float32)


# revision 18
# speedup vs baseline: 1.3163x; 1.1088x over previous
"""Trainium2 Bass kernel for the vq_codebook problem.

Computes, per batch b (B=32, d=512, n=4096, r=64, T=10, 3 steps):
    D = normalize(D_init, dim=d)
    repeat 3x: Dn = normalize(D); cos = Dn^T @ normalize(X, dim=d);
               C = softmax(cos / T, over r); D = X @ C^T   (normalize-invariant
               scale factors like the per-codeword count division cancel)
    Xbar = normalize(D) @ C of the last step.

Sharding: pure batch parallelism, 4 batches per NeuronCore across 8 cores.

Layout strategy (per core):
  - Host uploads X twice in the layouts the PE wants: XT = X^T in bf16
    (contraction over n for X@C^T, also the 1/||x_n|| source) and X8 = X
    in fp8-e4m3 packed in d-tile pairs (stationary operand of the cos
    matmul, DoubleRow mode).  No on-device transposes or casts of X.
  - n is chunked p-major (n = p*32 + c): chunk c covers n = {p*32+c},
    so every per-n quantity (logit scale, softmax denominator) is a
    per-partition vector for that chunk.
  - cos^T lands in [n-partitions, r-free] via fp8 DoubleRow matmuls with
    full 128-partition outputs; the softmax pipeline is emitted per
    8-chunk PSUM bank (logits on GPSIMD, exp on ACT, sum/recip/scale on
    DVE at fast-mode rates); X@C^T runs in the natural [d-partitions, r]
    layout, then a cheap bf16 transpose gives D^T for the normalize.
  - Two batches are software-pipelined at sub-step (bank) granularity so
    every engine queue interleaves two independent dependency chains.
  - Y is produced in bf16 (copies split ACT/GPSIMD, 4 tiles per DMA) and
    upcast to f32 on the host after the gather.
"""

import contextlib
import math

import numpy as np

import concourse.bacc as bacc
import concourse.bass as bass
import concourse.mybir as mybir
import concourse.tile as tile
from concourse.bass_utils import run_bass_kernel_spmd

F32 = mybir.dt.float32
BF16 = mybir.dt.bfloat16
F8 = mybir.dt.float8e4
AF = mybir.ActivationFunctionType
OP = mybir.AluOpType
DR = mybir.MatmulPerfMode.DoubleRow

N_CORES = 8
B_FULL, D, N, R = 32, 512, 4096, 64
B_LOC = B_FULL // N_CORES          # 4 batches per core
KT = D // 128                      # 4 d-tiles
NC = N // 128                      # 32 n-chunks of 128 (p-major: n=p*32+c)
T_INV = 0.1                        # 1 / temperature
LN_TINV = math.log(T_INV)
STEPS = 3
EPS2 = 1e-12                       # eps^2 for the norm clamp


def _rsqrt_clamped(nc, pool, src_ap, p, name, eps_t, bias2=None):
    """exp(-0.5 * ln(src + EPS2) [+ bias2]) as a [p, 1] tile."""
    m = src_ap.shape[1]
    ln = pool.tile([p, m], F32, tag=f"{name}_ln")
    nc.scalar.activation(out=ln, in_=src_ap, func=AF.Ln, scale=1.0,
                         bias=eps_t[:p, 0:1])
    rs = pool.tile([p, m], F32, tag=f"{name}_rs")
    nc.scalar.activation(out=rs, in_=ln, func=AF.Exp, scale=-0.5,
                         bias=0.0 if bias2 is None else bias2[:p, 0:1])
    return rs


def _force_single_act_set():
    """All ACT functions we use (Exp, Ln, Square, Copy) live in the
    natural_log_exp_and_others set.  Empty out every other set so a single
    table load suffices."""
    import concourse.hw_specs as hw_specs

    orig = hw_specs.get_activation_tables
    target = "natural_log_exp_and_others"

    def patched(arch):
        t = dict(orig(arch))
        need = {AF.Exp, AF.Ln, AF.Square, AF.Copy}
        if target in t and need <= set(t[target]):
            t = {k: (v if k == target else set()) for k, v in t.items()}
        return t

    bacc.get_activation_tables = patched


def build_program():
    _force_single_act_set()
    nc = bacc.Bacc()
    # X^T, p-major n rows: XT[b, p, c, d] = X[b, d, p*32+c], bf16
    xt_ext = nc.declare_dram_parameter("XT", [B_LOC, 128, NC, D], BF16,
                                       isOutput=False)
    # X natural fp8, d-tiles packed in pairs: [kp, p, t, n], d=kp*256+t*128+p
    x8_ext = nc.declare_dram_parameter("X8", [B_LOC, 2, 128, 2, N], F8,
                                       isOutput=False)
    # D_init^T: [r, d] bf16 (host pre-transposed)
    dt_ext = nc.declare_dram_parameter("DT", [B_LOC, R, D], BF16,
                                       isOutput=False)
    id_ext = nc.declare_dram_parameter("ident", [128, 128], BF16,
                                       isOutput=False)
    y_ext = nc.declare_dram_parameter("Y", [B_LOC, D, N], BF16, isOutput=True)

    with tile.TileContext(nc) as tc:
        with contextlib.ExitStack() as ctx:
            singles = ctx.enter_context(tc.tile_pool(name="singles", bufs=1))
            xpool = ctx.enter_context(tc.tile_pool(name="xpool", bufs=2))
            work = ctx.enter_context(tc.tile_pool(name="work", bufs=2))
            ypool = ctx.enter_context(tc.tile_pool(name="ypool", bufs=4))
            ps_cos = ctx.enter_context(
                tc.tile_pool(name="ps_cos", bufs=2, space="PSUM"))
            ps_m = ctx.enter_context(
                tc.tile_pool(name="ps_m", bufs=2, space="PSUM"))
            ps_out = ctx.enter_context(
                tc.tile_pool(name="ps_out", bufs=2, space="PSUM"))

            id_b = singles.tile([128, 128], BF16)
            nc.sync.dma_start(out=id_b, in_=id_ext[:])
            eps_t = singles.tile([128, 1], F32)
            nc.vector.memset(eps_t, EPS2)
            lnt_t = singles.tile([128, 1], F32)
            nc.vector.memset(lnt_t, LN_TINV)

            def emit_loads(b):
                """Input DMAs for batch b; returns a state dict."""
                xt = xpool.tile([128, NC, D], BF16, tag="xt", name=f"xt{b}",
                                bufs=3)
                nc.sync.dma_start(out=xt, in_=xt_ext[b, :, :, :])
                x8 = []
                for kp in range(2):
                    t = xpool.tile([128, 2, N], F8, tag=f"x8_{kp}",
                                   name=f"x8_{b}_{kp}")
                    nc.sync.dma_start(out=t, in_=x8_ext[b, kp])
                    x8.append(t)
                dt0 = xpool.tile([64, D], BF16, tag="dt0", name=f"dt0_{b}")
                nc.sync.dma_start(out=dt0, in_=dt_ext[b])
                return {"xt": xt, "x8": x8, "dt": dt0}

            def ph_prep_q(st, q):
                """ssq for chunks 8q..8q+7 (DVE: one 2x square + 8 4x
                accumulates); after the last quarter, the scl2 chain."""
                xt = st["xt"]
                if q == 0:
                    st["ssq"] = work.tile([128, NC], F32, tag="ssq", name="ssq")
                    st["sq_scr"] = work.tile([128, 8, D], BF16, tag="sqscr", name="sqscr", bufs=1)
                ssq, sq_scr = st["ssq"], st["sq_scr"]
                nc.vector.tensor_tensor(
                    out=sq_scr, in0=xt[:, 8 * q:8 * (q + 1), :],
                    in1=xt[:, 8 * q:8 * (q + 1), :], op=OP.mult,
                )
                for j in range(8):
                    c = 8 * q + j
                    nc.vector.tensor_scalar(
                        out=sq_scr[:, j, :], in0=sq_scr[:, j, :],
                        scalar1=1.0, scalar2=None, op0=OP.mult,
                        accum_out=ssq[:, c:c + 1],
                    )
                if q == 3:
                    ln_x = work.tile([128, NC], F32, tag="lnx")
                    nc.scalar.activation(out=ln_x, in_=ssq, func=AF.Ln,
                                         scale=1.0, bias=eps_t[:, 0:1])
                    scl2 = work.tile([128, NC], F32, tag="scl2")
                    nc.scalar.activation(out=scl2, in_=ln_x, func=AF.Exp,
                                         scale=-0.5, bias=lnt_t[:, 0:1])
                    st["scl2"] = scl2

            def ph_dchain(st, s):
                """Normalize D columns (rows of D^T) -> dn8 fp8 natural;
                allocate the step's softmax tiles."""
                dt_cur = st["dt"]
                ssqd = work.tile([64, 1], F32, tag="ssqd")
                dsq_scr = work.tile([64, D], BF16, tag="dsqscr", bufs=1)
                nc.scalar.activation(
                    out=dsq_scr, in_=dt_cur, func=AF.Square, scale=1.0,
                    bias=0.0, accum_out=ssqd,
                )
                rnd = _rsqrt_clamped(nc, work, ssqd[:, :], 64, "rnd", eps_t)
                dnt = work.tile([64, D], BF16, tag="dnt")
                nc.scalar.activation(out=dnt, in_=dt_cur, func=AF.Copy,
                                     scale=rnd[:, 0:1])
                pdn = ps_m.tile([128, KT, R], BF16, tag="psmall")
                for k in range(KT):
                    nc.tensor.transpose(
                        pdn[:, k, :], dnt[:, k * 128:(k + 1) * 128],
                        id_b[0:64, 0:64])
                dn8 = work.tile([128, KT, R], F8, tag="dn8")
                nc.vector.tensor_copy(out=dn8, in_=pdn)
                st["dn8"] = dn8
                st["lg"] = work.tile([128, NC, R], BF16, tag="lg", name="lg")
                st["et"] = work.tile([128, NC, R], BF16, tag="et", name="et")
                st["ct"] = work.tile([128, NC, R], BF16, tag="ct", name="ct")
                st["ssum"] = work.tile([128, NC], F32, tag="ssum", name="ssum")
                st["rs"] = work.tile([128, NC], F32, tag="rs", name="rs")

            def ph_bank(st, g):
                """One 8-chunk softmax bank: cos (PE, fp8 DoubleRow) ->
                logits (GPSIMD) -> exp (ACT) -> sum (DVE) -> 1/s (DVE) ->
                ct = et/s (DVE 4x per chunk)."""
                x8, dn8, scl2 = st["x8"], st["dn8"], st["scl2"]
                lg, et, ct = st["lg"], st["et"], st["ct"]
                s_sum, rs = st["ssum"], st["rs"]
                pc = ps_cos.tile([128, 8, R], F32, tag="pcos")
                for j in range(8):
                    c = g * 8 + j
                    for kp in range(2):
                        t = x8[kp]
                        x8col = bass.AP(
                            tensor=t.tensor, offset=t.offset + c,
                            ap=[list(t.ap[0]), [N, 2], [NC, 128]],
                        )
                        nc.tensor.matmul(
                            pc[:, j, :], x8col,
                            dn8[:, 2 * kp:2 * kp + 2, :],
                            start=(kp == 0), stop=(kp == 1),
                            perf_mode=DR,
                        )
                scl_b = bass.AP(
                    tensor=scl2.tensor, offset=scl2.offset + 8 * g,
                    ap=[list(scl2.ap[0]), [1, 8], [0, R]],
                )
                nc.gpsimd.tensor_tensor(
                    out=lg[:, 8 * g:8 * (g + 1), :], in0=pc, in1=scl_b,
                    op=OP.mult,
                )
                nc.scalar.activation(
                    out=et[:, 8 * g:8 * (g + 1), :],
                    in_=lg[:, 8 * g:8 * (g + 1), :],
                    func=AF.Exp, scale=1.0, bias=0.0)
                nc.vector.tensor_reduce(
                    out=s_sum[:, 8 * g:8 * (g + 1)],
                    in_=et[:, 8 * g:8 * (g + 1), :],
                    axis=mybir.AxisListType.X, op=OP.add)
                nc.vector.reciprocal(
                    out=rs[:, 8 * g:8 * (g + 1)],
                    in_=s_sum[:, 8 * g:8 * (g + 1)])
                for j in range(8):
                    c = g * 8 + j
                    nc.vector.tensor_scalar(
                        out=ct[:, c, :], in0=et[:, c, :],
                        scalar1=rs[:, c:c + 1], scalar2=None, op0=OP.mult,
                    )

            def ph_tail(st, s):
                """XCt = X @ C^T (natural layout), transpose to D^T; on the
                final step also normalize into Dnew."""
                xt, ct = st["xt"], st["ct"]
                last = s == STEPS - 1
                pxct = ps_m.tile([128, KT, R], F32, tag="psmall")
                for dd in range(KT):
                    for c in range(NC):
                        nc.tensor.matmul(
                            pxct[:, dd, :],
                            xt[:, c, dd * 128:(dd + 1) * 128],
                            ct[:, c, :],
                            start=(c == 0), stop=(c == NC - 1),
                        )
                xct_n = work.tile([128, KT, R], BF16, tag="xctn")
                nc.vector.tensor_copy(out=xct_n, in_=pxct)
                pdt = ps_m.tile([64, D], BF16, tag="pdt")
                for dd in range(KT):
                    nc.tensor.transpose(
                        pdt[:, dd * 128:(dd + 1) * 128], xct_n[:, dd, :],
                        id_b)
                if not last:
                    st["dt"] = pdt
                else:
                    st["ct_last"] = ct
                    ssqf = work.tile([64, 1], F32, tag="ssqf")
                    fsq_scr = work.tile([64, D], BF16, tag="fsqscr", bufs=1)
                    nc.scalar.activation(
                        out=fsq_scr, in_=pdt, func=AF.Square, scale=1.0,
                        bias=0.0, accum_out=ssqf,
                    )
                    rnf = _rsqrt_clamped(nc, work, ssqf[:, :], 64, "rnf",
                                         eps_t)
                    dnew_b = work.tile([64, D], BF16, tag="dnewb")
                    nc.scalar.activation(out=dnew_b, in_=pdt, func=AF.Copy,
                                         scale=rnf[:, 0:1])
                    st["dnew"] = dnew_b

            def ph_cr(st, h):
                """C natural [r, n] from ct chunks (2 transpose groups)."""
                ct = st["ct_last"]
                if h == 0:
                    st["c_r"] = work.tile([64, N], BF16, tag="c_r", name="c_r")
                c_r = st["c_r"]
                for q in (2 * h, 2 * h + 1):
                    pcr = ps_cos.tile([64, 8, 128], BF16, tag="pcos")
                    for j in range(8):
                        nc.tensor.transpose(
                            pcr[:, j, :], ct[:, q * 8 + j, :], id_b)
                    dst = bass.AP(
                        tensor=c_r.tensor, offset=c_r.offset + 8 * q,
                        ap=[list(c_r.ap[0]), [1, 8], [NC, 128]],
                    )
                    nc.scalar.copy(out=dst, in_=pcr)

            def ph_y(st, b, u):
                """One output unit: 4 Xbar matmuls -> bf16 staging -> DMA."""
                c_r, dnew_b = st["c_r"], st["dnew"]
                k, half = u // 2, u % 2
                ot = ypool.tile([128, 4, 512], BF16, tag="osb")
                for q in range(4):
                    j = half * 4 + q
                    pxb = ps_out.tile([128, 512], F32, tag="pxb")
                    nc.tensor.matmul(
                        pxb, dnew_b[:, k * 128:(k + 1) * 128],
                        c_r[:, j * 512:(j + 1) * 512],
                        start=True, stop=True,
                    )
                    if u % 8 < 5:
                        nc.scalar.copy(out=ot[:, q, :], in_=pxb)
                    else:
                        nc.gpsimd.tensor_copy(out=ot[:, q, :], in_=pxb)
                nc.sync.dma_start(
                    out=y_ext[b, k * 128:(k + 1) * 128,
                              half * 2048:(half + 1) * 2048],
                    in_=ot,
                )

            def batch_phases(b, st, prefetch):
                """The full phase list for one batch."""
                ph = []
                ph.append(lambda: ph_dchain(st, 0))
                for q in range(4):
                    ph.append(lambda q=q: ph_prep_q(st, q))
                for g in range(4):
                    ph.append(lambda g=g: ph_bank(st, g))
                ph.append(lambda: (ph_tail(st, 0), prefetch()))
                for s in (1, 2):
                    ph.append(lambda s=s: ph_dchain(st, s))
                    for g in range(4):
                        ph.append(lambda g=g: ph_bank(st, g))
                    ph.append(lambda s=s: ph_tail(st, s))
                for h in range(2):
                    ph.append(lambda h=h: ph_cr(st, h))
                for u in range(8):
                    ph.append(lambda u=u: ph_y(st, b, u))
                return ph

            # ---- two-batch software pipeline --------------------------------
            states = {0: emit_loads(0), 1: emit_loads(1)}
            for pair0 in range(0, B_LOC, 2):
                pair = [pair0, pair0 + 1]

                def make_prefetch(pair0=pair0):
                    done = {"v": False}

                    def prefetch():
                        if not done["v"] and pair0 + 2 < B_LOC:
                            done["v"] = True
                            states[pair0 + 2] = emit_loads(pair0 + 2)
                            states[pair0 + 3] = emit_loads(pair0 + 3)

                    return prefetch

                pf = make_prefetch()
                phs = [batch_phases(b, states[b], pf) for b in pair]
                for p0, p1 in zip(phs[0], phs[1]):
                    p0()
                    p1()
                for b in pair:
                    del states[b]
    nc.finalize()
    return nc


_NC_CACHE = None
_last_in_maps = None


def kernel(X: np.ndarray, D_init: np.ndarray) -> np.ndarray:
    global _NC_CACHE, _last_in_maps
    import ml_dtypes

    X = np.asarray(X, dtype=np.float32)
    D_init = np.asarray(D_init, dtype=np.float32)
    if _NC_CACHE is None:
        _NC_CACHE = build_program()
    nc = _NC_CACHE
    ident = np.eye(128, dtype=ml_dtypes.bfloat16)
    # XT[b, p, c, d] = X[b, d, p*32+c]
    xt_h = np.ascontiguousarray(
        X.transpose(0, 2, 1).reshape(B_FULL, 128, NC, D)
    ).astype(ml_dtypes.bfloat16)
    # X8[b, kp, p, t, n] = X[b, kp*256+t*128+p, n]
    x8_h = np.ascontiguousarray(
        X.reshape(B_FULL, 2, 2, 128, N).transpose(0, 1, 3, 2, 4)
    ).astype(ml_dtypes.float8_e4m3)
    dt_h = np.ascontiguousarray(
        D_init.transpose(0, 2, 1)).astype(ml_dtypes.bfloat16)
    in_maps = [
        {
            "XT": np.ascontiguousarray(xt_h[i * B_LOC:(i + 1) * B_LOC]),
            "X8": np.ascontiguousarray(x8_h[i * B_LOC:(i + 1) * B_LOC]),
            "DT": np.ascontiguousarray(dt_h[i * B_LOC:(i + 1) * B_LOC]),
            "ident": ident,
        }
        for i in range(N_CORES)
    ]
    _last_in_maps = in_maps
    res = run_bass_kernel_spmd(nc, in_maps, list(range(N_CORES)))
    out = np.concatenate(
        [np.asarray(res.results[i]["Y"]) for i in range(N_CORES)], axis=0)
    return out.astype(np.float32)


# revision 20
# speedup vs baseline: 1.5122x; 1.1488x over previous
"""Trainium2 Bass kernel for the vq_codebook problem.

Computes, per batch b (B=32, d=512, n=4096, r=64, T=10, 3 steps):
    D = normalize(D_init, dim=d)
    repeat 3x: Dn = normalize(D); cos = Dn^T @ normalize(X, dim=d);
               C = softmax(cos / T, over r); D = X @ C^T   (normalize-invariant
               scale factors like the per-codeword count division cancel)
    Xbar = normalize(D) @ C of the last step.

Sharding: pure batch parallelism, 4 batches per NeuronCore across 8 cores.

Layout strategy (per core):
  - Host uploads X twice in the layouts the PE wants: XT = X^T in bf16
    (contraction over n for X@C^T, also the 1/||x_n|| source) and X8 = X
    in fp8-e4m3 packed in d-tile pairs (stationary operand of the cos
    matmul, DoubleRow mode).  No on-device transposes or casts of X.
  - n is chunked p-major (n = p*32 + c): chunk c covers n = {p*32+c},
    so every per-n quantity (logit scale, softmax denominator) is a
    per-partition vector for that chunk.
  - cos^T lands in [n-partitions, r-free] via fp8 DoubleRow matmuls with
    full 128-partition outputs; the softmax pipeline is emitted per
    8-chunk PSUM bank (logits on GPSIMD, exp on ACT, sum/recip/scale on
    DVE at fast-mode rates); X@C^T runs in the natural [d-partitions, r]
    layout, then a cheap bf16 transpose gives D^T for the normalize.
  - Two batches are software-pipelined at sub-step (bank) granularity so
    every engine queue interleaves two independent dependency chains.
  - Y is produced in bf16 (copies split ACT/GPSIMD, 4 tiles per DMA) and
    upcast to f32 on the host after the gather.
"""

import contextlib
import math

import numpy as np

import concourse.bacc as bacc
import concourse.bass as bass
import concourse.mybir as mybir
import concourse.tile as tile
from concourse.bass_utils import run_bass_kernel_spmd

F32 = mybir.dt.float32
BF16 = mybir.dt.bfloat16
F8 = mybir.dt.float8e4
AF = mybir.ActivationFunctionType
OP = mybir.AluOpType
DR = mybir.MatmulPerfMode.DoubleRow

N_CORES = 8
B_FULL, D, N, R = 32, 512, 4096, 64
B_LOC = B_FULL // N_CORES          # 4 batches per core
KT = D // 128                      # 4 d-tiles
NC = N // 128                      # 32 n-chunks of 128 (p-major: n=p*32+c)
T_INV = 0.1                        # 1 / temperature
LN_TINV = math.log(T_INV)
STEPS = 3
EPS2 = 1e-12                       # eps^2 for the norm clamp


def _rsqrt_clamped(nc, pool, src_ap, p, name, eps_t, bias2=None):
    """exp(-0.5 * ln(src + EPS2) [+ bias2]) as a [p, 1] tile."""
    m = src_ap.shape[1]
    ln = pool.tile([p, m], F32, tag=f"{name}_ln")
    nc.scalar.activation(out=ln, in_=src_ap, func=AF.Ln, scale=1.0,
                         bias=eps_t[:p, 0:1])
    rs = pool.tile([p, m], F32, tag=f"{name}_rs")
    nc.scalar.activation(out=rs, in_=ln, func=AF.Exp, scale=-0.5,
                         bias=0.0 if bias2 is None else bias2[:p, 0:1])
    return rs


def _force_single_act_set():
    """All ACT functions we use (Exp, Ln, Square, Copy) live in the
    natural_log_exp_and_others set.  Empty out every other set so a single
    table load suffices."""
    import concourse.hw_specs as hw_specs

    orig = hw_specs.get_activation_tables
    target = "natural_log_exp_and_others"

    def patched(arch):
        t = dict(orig(arch))
        need = {AF.Exp, AF.Ln, AF.Square, AF.Copy}
        if target in t and need <= set(t[target]):
            t = {k: (v if k == target else set()) for k, v in t.items()}
        return t

    bacc.get_activation_tables = patched


def build_program():
    _force_single_act_set()
    nc = bacc.Bacc()
    # X^T, p-major n rows: XT[b, p, c, d] = X[b, d, p*32+c], bf16
    xt_ext = nc.declare_dram_parameter("XT", [B_LOC, 128, NC, D], BF16,
                                       isOutput=False)
    # X natural fp8, d-tiles packed in pairs: [kp, p, t, n], d=kp*256+t*128+p
    x8_ext = nc.declare_dram_parameter("X8", [B_LOC, 2, 128, 2, N], F8,
                                       isOutput=False)
    # D_init^T: [r, d] bf16 (host pre-transposed)
    dt_ext = nc.declare_dram_parameter("DT", [B_LOC, R, D], BF16,
                                       isOutput=False)
    id_ext = nc.declare_dram_parameter("ident", [128, 128], BF16,
                                       isOutput=False)
    y_ext = nc.declare_dram_parameter("Y", [B_LOC, D, N], BF16, isOutput=True)

    with tile.TileContext(nc) as tc:
        with contextlib.ExitStack() as ctx:
            singles = ctx.enter_context(tc.tile_pool(name="singles", bufs=1))
            xpool = ctx.enter_context(tc.tile_pool(name="xpool", bufs=2))
            work = ctx.enter_context(tc.tile_pool(name="work", bufs=2))
            ypool = ctx.enter_context(tc.tile_pool(name="ypool", bufs=4))
            ps_cos = ctx.enter_context(
                tc.tile_pool(name="ps_cos", bufs=2, space="PSUM"))
            ps_m = ctx.enter_context(
                tc.tile_pool(name="ps_m", bufs=2, space="PSUM"))
            ps_out = ctx.enter_context(
                tc.tile_pool(name="ps_out", bufs=2, space="PSUM"))

            id_b = singles.tile([128, 128], BF16)
            nc.sync.dma_start(out=id_b, in_=id_ext[:])
            eps_t = singles.tile([128, 1], F32)
            nc.vector.memset(eps_t, EPS2)
            lnt_t = singles.tile([128, 1], F32)
            nc.vector.memset(lnt_t, LN_TINV)

            def emit_loads(b):
                """Input DMAs for batch b; returns a state dict."""
                xt = xpool.tile([128, NC, D], BF16, tag="xt", name=f"xt{b}",
                                bufs=3)
                nc.sync.dma_start(out=xt, in_=xt_ext[b, :, :, :])
                x8 = []
                for kp in range(2):
                    t = xpool.tile([128, 2, N], F8, tag=f"x8_{kp}",
                                   name=f"x8_{b}_{kp}")
                    nc.sync.dma_start(out=t, in_=x8_ext[b, kp])
                    x8.append(t)
                dt0 = xpool.tile([64, D], BF16, tag="dt0", name=f"dt0_{b}")
                nc.sync.dma_start(out=dt0, in_=dt_ext[b])
                return {"xt": xt, "x8": x8, "dt": dt0}

            def ph_prep_q(st, q):
                """ssq for chunks 8q..8q+7 (DVE: one 2x square + 8 4x
                accumulates); after the last quarter, the scl2 chain."""
                xt = st["xt"]
                if q == 0:
                    st["ssq"] = work.tile([128, NC], F32, tag="ssq", name="ssq")
                    st["sq_scr"] = work.tile([128, 8, D], BF16, tag="sqscr", name="sqscr", bufs=1)
                ssq, sq_scr = st["ssq"], st["sq_scr"]
                nc.vector.tensor_tensor(
                    out=sq_scr, in0=xt[:, 8 * q:8 * (q + 1), :],
                    in1=xt[:, 8 * q:8 * (q + 1), :], op=OP.mult,
                )
                for j in range(8):
                    c = 8 * q + j
                    nc.vector.tensor_scalar(
                        out=sq_scr[:, j, :], in0=sq_scr[:, j, :],
                        scalar1=1.0, scalar2=None, op0=OP.mult,
                        accum_out=ssq[:, c:c + 1],
                    )
                if q == 3:
                    ln_x = work.tile([128, NC], F32, tag="lnx")
                    nc.scalar.activation(out=ln_x, in_=ssq, func=AF.Ln,
                                         scale=1.0, bias=eps_t[:, 0:1])
                    scl2 = work.tile([128, NC], F32, tag="scl2")
                    nc.scalar.activation(out=scl2, in_=ln_x, func=AF.Exp,
                                         scale=-0.5, bias=lnt_t[:, 0:1])
                    st["scl2"] = scl2

            def ph_dchain(st, s):
                """Normalize D columns (rows of D^T) -> dn8 fp8 natural;
                allocate the step's softmax tiles."""
                dt_cur = st["dt"]
                ssqd = work.tile([64, 1], F32, tag="ssqd")
                dsq_scr = work.tile([64, D], BF16, tag="dsqscr", bufs=1)
                nc.scalar.activation(
                    out=dsq_scr, in_=dt_cur, func=AF.Square, scale=1.0,
                    bias=0.0, accum_out=ssqd,
                )
                rnd = _rsqrt_clamped(nc, work, ssqd[:, :], 64, "rnd", eps_t)
                dnt = work.tile([64, D], BF16, tag="dnt")
                nc.scalar.activation(out=dnt, in_=dt_cur, func=AF.Copy,
                                     scale=rnd[:, 0:1])
                pdn = ps_m.tile([128, KT, R], BF16, tag="psmall")
                for k in range(KT):
                    nc.tensor.transpose(
                        pdn[:, k, :], dnt[:, k * 128:(k + 1) * 128],
                        id_b[0:64, 0:64])
                dn8 = work.tile([128, KT, R], F8, tag="dn8", bufs=3)
                nc.vector.tensor_copy(out=dn8, in_=pdn)
                st["dn8"] = dn8
                st["et"] = work.tile([128, NC, R], BF16, tag="et", name="et",
                                     bufs=3)
                st["ct"] = work.tile([128, NC, R], BF16, tag="ct", name="ct",
                                     bufs=3)
                st["ssum"] = work.tile([128, NC], F32, tag="ssum", name="ssum",
                                       bufs=3)
                st["rs"] = work.tile([128, NC], F32, tag="rs", name="rs",
                                     bufs=3)

            def ph_bank(st, g):
                """One 8-chunk softmax bank: cos (PE, fp8 DoubleRow) ->
                logits (GPSIMD) -> exp (ACT) -> sum (DVE) -> 1/s (DVE) ->
                ct = et/s (DVE 4x per chunk)."""
                x8, dn8, scl2 = st["x8"], st["dn8"], st["scl2"]
                et, ct = st["et"], st["ct"]
                s_sum, rs = st["ssum"], st["rs"]
                pc = ps_cos.tile([128, 8, R], F32, tag="pcos")
                for j in range(8):
                    c = g * 8 + j
                    for kp in range(2):
                        t = x8[kp]
                        x8col = bass.AP(
                            tensor=t.tensor, offset=t.offset + c,
                            ap=[list(t.ap[0]), [N, 2], [NC, 128]],
                        )
                        nc.tensor.matmul(
                            pc[:, j, :], x8col,
                            dn8[:, 2 * kp:2 * kp + 2, :],
                            start=(kp == 0), stop=(kp == 1),
                            perf_mode=DR,
                        )
                scl_b = bass.AP(
                    tensor=scl2.tensor, offset=scl2.offset + 8 * g,
                    ap=[list(scl2.ap[0]), [1, 8], [0, R]],
                )
                nc.gpsimd.tensor_tensor(
                    out=et[:, 8 * g:8 * (g + 1), :], in0=pc, in1=scl_b,
                    op=OP.mult,
                )
                nc.scalar.activation(
                    out=et[:, 8 * g:8 * (g + 1), :],
                    in_=et[:, 8 * g:8 * (g + 1), :],
                    func=AF.Exp, scale=1.0, bias=0.0)
                nc.vector.tensor_reduce(
                    out=s_sum[:, 8 * g:8 * (g + 1)],
                    in_=et[:, 8 * g:8 * (g + 1), :],
                    axis=mybir.AxisListType.X, op=OP.add)
                nc.vector.reciprocal(
                    out=rs[:, 8 * g:8 * (g + 1)],
                    in_=s_sum[:, 8 * g:8 * (g + 1)])
                for j in range(8):
                    c = g * 8 + j
                    nc.vector.tensor_scalar(
                        out=ct[:, c, :], in0=et[:, c, :],
                        scalar1=rs[:, c:c + 1], scalar2=None, op0=OP.mult,
                    )

            def ph_tail(st, s):
                """XCt = X @ C^T (natural layout), transpose to D^T; on the
                final step also normalize into Dnew."""
                xt, ct = st["xt"], st["ct"]
                last = s == STEPS - 1
                pxct = ps_m.tile([128, KT, R], F32, tag="psmall")
                for dd in range(KT):
                    for c in range(NC):
                        nc.tensor.matmul(
                            pxct[:, dd, :],
                            xt[:, c, dd * 128:(dd + 1) * 128],
                            ct[:, c, :],
                            start=(c == 0), stop=(c == NC - 1),
                        )
                xct_n = work.tile([128, KT, R], BF16, tag="xctn")
                nc.vector.tensor_copy(out=xct_n, in_=pxct)
                pdt = ps_m.tile([64, D], BF16, tag="pdt")
                for dd in range(KT):
                    nc.tensor.transpose(
                        pdt[:, dd * 128:(dd + 1) * 128], xct_n[:, dd, :],
                        id_b)
                if not last:
                    st["dt"] = pdt
                else:
                    st["ct_last"] = ct
                    ssqf = work.tile([64, 1], F32, tag="ssqf")
                    fsq_scr = work.tile([64, D], BF16, tag="fsqscr", bufs=1)
                    nc.scalar.activation(
                        out=fsq_scr, in_=pdt, func=AF.Square, scale=1.0,
                        bias=0.0, accum_out=ssqf,
                    )
                    rnf = _rsqrt_clamped(nc, work, ssqf[:, :], 64, "rnf",
                                         eps_t)
                    dnew_b = work.tile([64, D], BF16, tag="dnewb")
                    nc.scalar.activation(out=dnew_b, in_=pdt, func=AF.Copy,
                                         scale=rnf[:, 0:1])
                    st["dnew"] = dnew_b

            def ph_cr(st, h):
                """C natural [r, n] from ct chunks (2 transpose groups)."""
                ct = st["ct_last"]
                if h == 0:
                    st["c_r"] = work.tile([64, N], BF16, tag="c_r", name="c_r")
                c_r = st["c_r"]
                for q in (2 * h, 2 * h + 1):
                    pcr = ps_cos.tile([64, 8, 128], BF16, tag="pcos")
                    for j in range(8):
                        nc.tensor.transpose(
                            pcr[:, j, :], ct[:, q * 8 + j, :], id_b)
                    dst = bass.AP(
                        tensor=c_r.tensor, offset=c_r.offset + 8 * q,
                        ap=[list(c_r.ap[0]), [1, 8], [NC, 128]],
                    )
                    nc.scalar.copy(out=dst, in_=pcr)

            def ph_y(st, b, u):
                """One output unit: 4 Xbar matmuls -> bf16 staging -> DMA."""
                c_r, dnew_b = st["c_r"], st["dnew"]
                k, half = u // 2, u % 2
                ot = ypool.tile([128, 4, 512], BF16, tag="osb")
                for q in range(4):
                    j = half * 4 + q
                    pxb = ps_out.tile([128, 512], F32, tag="pxb")
                    nc.tensor.matmul(
                        pxb, dnew_b[:, k * 128:(k + 1) * 128],
                        c_r[:, j * 512:(j + 1) * 512],
                        start=True, stop=True,
                    )
                    e = (u * 4 + q) % 3
                    if e == 0:
                        nc.scalar.copy(out=ot[:, q, :], in_=pxb)
                    elif e == 1:
                        nc.gpsimd.tensor_copy(out=ot[:, q, :], in_=pxb)
                    else:
                        nc.vector.tensor_copy(out=ot[:, q, :], in_=pxb)
                nc.sync.dma_start(
                    out=y_ext[b, k * 128:(k + 1) * 128,
                              half * 2048:(half + 1) * 2048],
                    in_=ot,
                )

            def batch_phases(b, st, prefetch):
                """The full phase list for one batch."""
                ph = []
                ph.append(lambda: ph_dchain(st, 0))
                for q in range(4):
                    ph.append(lambda q=q: ph_prep_q(st, q))
                for g in range(4):
                    ph.append(lambda g=g: ph_bank(st, g))
                ph.append(lambda: (ph_tail(st, 0), prefetch()))
                for s in (1, 2):
                    ph.append(lambda s=s: ph_dchain(st, s))
                    for g in range(4):
                        ph.append(lambda g=g: ph_bank(st, g))
                    ph.append(lambda s=s: ph_tail(st, s))
                for h in range(2):
                    ph.append(lambda h=h: ph_cr(st, h))
                for u in range(8):
                    ph.append(lambda u=u: ph_y(st, b, u))
                return ph

            # ---- skewed 4-batch software pipeline ---------------------------
            OFFS = [0, 6, 16, 22]      # slot offset per batch
            NPH = 32                   # phases per batch
            LOAD_AHEAD = 3             # emit loads this many slots early
            states = {}
            phases = {}

            def start_batch(b):
                states[b] = emit_loads(b)
                st = states[b]
                ph = [lambda: ph_dchain(st, 0)]
                for q in range(4):
                    ph.append(lambda q=q: ph_prep_q(st, q))
                for g in range(4):
                    ph.append(lambda g=g: ph_bank(st, g))
                ph.append(lambda: ph_tail(st, 0))
                for s in (1, 2):
                    ph.append(lambda s=s: ph_dchain(st, s))
                    for g in range(4):
                        ph.append(lambda g=g: ph_bank(st, g))
                    ph.append(lambda s=s: ph_tail(st, s))
                for h in range(2):
                    ph.append(lambda h=h: ph_cr(st, h))
                for u in range(8):
                    ph.append(lambda u=u: ph_y(st, b, u))
                assert len(ph) == NPH
                phases[b] = ph

            for t in range(0, OFFS[-1] + NPH):
                for b in range(B_LOC):
                    if t == max(0, OFFS[b] - LOAD_AHEAD) and b not in phases:
                        start_batch(b)
                # emit earlier-stage pipes first (prep/step work ahead of
                # output copies in every engine queue)
                active = [b for b in range(B_LOC)
                          if b in phases and 0 <= t - OFFS[b] < NPH]
                for b in sorted(active, key=lambda b: t - OFFS[b]):
                    phases[b][t - OFFS[b]]()
    nc.finalize()
    return nc


_NC_CACHE = None
_last_in_maps = None


def kernel(X: np.ndarray, D_init: np.ndarray) -> np.ndarray:
    global _NC_CACHE, _last_in_maps
    import ml_dtypes

    X = np.asarray(X, dtype=np.float32)
    D_init = np.asarray(D_init, dtype=np.float32)
    if _NC_CACHE is None:
        _NC_CACHE = build_program()
    nc = _NC_CACHE
    ident = np.eye(128, dtype=ml_dtypes.bfloat16)
    # XT[b, p, c, d] = X[b, d, p*32+c]
    xt_h = np.ascontiguousarray(
        X.transpose(0, 2, 1).reshape(B_FULL, 128, NC, D)
    ).astype(ml_dtypes.bfloat16)
    # X8[b, kp, p, t, n] = X[b, kp*256+t*128+p, n]
    x8_h = np.ascontiguousarray(
        X.reshape(B_FULL, 2, 2, 128, N).transpose(0, 1, 3, 2, 4)
    ).astype(ml_dtypes.float8_e4m3)
    dt_h = np.ascontiguousarray(
        D_init.transpose(0, 2, 1)).astype(ml_dtypes.bfloat16)
    in_maps = [
        {
            "XT": np.ascontiguousarray(xt_h[i * B_LOC:(i + 1) * B_LOC]),
            "X8": np.ascontiguousarray(x8_h[i * B_LOC:(i + 1) * B_LOC]),
            "DT": np.ascontiguousarray(dt_h[i * B_LOC:(i + 1) * B_LOC]),
            "ident": ident,
        }
        for i in range(N_CORES)
    ]
    _last_in_maps = in_maps
    res = run_bass_kernel_spmd(nc, in_maps, list(range(N_CORES)))
    out = np.concatenate(
        [np.asarray(res.results[i]["Y"]) for i in range(N_CORES)], axis=0)
    return out.astype(np.float32)
